# revision 48
# baseline (speedup 1.0000x reference)
"""Trainium2 Bass kernel for nn_KnowledgeDifficulty.

Math (per batch b):
  logits = X[b] @ Wa            (N, M)   (ba==0 and cancels in softmax anyway)
  w      = softmax(logits, axis=N)
  d      = sigmoid((sum_n e[n,m] * y[n]) / (sum_n e[n,m]) + bs)
    where e = exp(logits), y = X[b] @ Ws
  out    = d * (K > 0)

v3 design (per core, 8 batches):
  - mm1 per (b, chunk): lhsT = xt chunk (stationary), stream waws -> lg PSUM.
  - exp of lg [128,1024] alternates between ACT (real Exp) and DVE
    (Schraudolph bit-trick: bf16 bits = round(x*128/ln2 + 16250.49), one
    tensor_scalar with int16 output aliasing the bf16 e tile). Last tile is
    column-split across both engines to cut the trailing latency.
  - mm2: col-tiled 4-way (tile_position), lhsT=[y|1] per batch (y computed
    host-side, tiny), accumulates t,s rows into out2 PSUM rows {32j,32j+1}.
  - t/s rows transposed to partition-parallel layout via 8 tiny PE matmuls
    per group against a 0/1 selector (lhsT = ts block bf16, rhs = sel).
  - epilogue: d*K = kh*(1+tanh(r/2 + bs/2)) with kh = 0.5*K (host-packed
    f32 bits in the i32 bnk tensor); tanh shares the exp ACT table set.
  - DMAs: xt staged b0|b1|b23|b45|b67 on the sync HWDGE ring; weights/y on
    the scalar ring; dummy exp up front to preload the ACT exp table.

Sharding: data-parallel over B across 8 cores. Output [128, g, (k,u)] f32;
host un-shuffles.
"""

import math

import numpy as np

B, N, L, M = 64, 512, 128, 1024
NCORES = 8
BLOC = B // NCORES  # 8 batches per core
NCH = N // 128  # 4 chunks of 128 along N
HALF = 512  # one PSUM bank of fp32
NGRP = 2  # two groups of 4 batches (4 PE column groups each)
GSZ = BLOC // NGRP  # 4
NBLK = M // 128  # 8 m-blocks of 128
SELC = 2 * GSZ  # 8 selector cols (t,s per batch in group)
WAW = M + 2 + SELC  # waws cols: Wa | Ws | pad | selector

# Schraudolph exp in bf16 bits: bits = round(x * 2^7/ln2 + (127*128 - C))
SCH_A = 128.0 / math.log(2.0)
SCH_B = 127.0 * 128.0 - 128.0 * math.log2(1.0615) / 2.0

_STATE = {}


def _dedupe_ldweights(nc, mybir):
    """Drop back-to-back InstLdweights with identical weights (the two mm1
    halves share a stationary operand; the PE keeps weights across matmuls)."""
    removed = 0
    for bb in nc.main_func.blocks:
        last_key = None
        keep = []
        for inst in bb.instructions:
            if isinstance(inst, mybir.InstLdweights):
                key = (
                    repr(inst.ins[0]),
                    getattr(inst, "tile_position", None),
                    getattr(inst, "perf_mode", None),
                    getattr(inst, "is_transpose", None),
                )
                if key == last_key:
                    removed += 1
                    nc.inst_map.pop(inst.name, None)
                    continue
                last_key = key
            keep.append(inst)
        bb.instructions[:] = keep
    return removed


def _build():
    import concourse.bacc as bacc
    import concourse.tile as tile
    import concourse.mybir as mybir

    f32 = mybir.dt.float32
    bf16 = mybir.dt.bfloat16
    i16 = mybir.dt.int16
    i32 = mybir.dt.int32
    Exp = mybir.ActivationFunctionType.Exp
    Tanh = mybir.ActivationFunctionType.Tanh
    MULT = mybir.AluOpType.mult
    ADD = mybir.AluOpType.add
    DIV = mybir.AluOpType.divide

    nc = bacc.Bacc(
        "TRN2", target_bir_lowering=False, debug=False, num_devices=NCORES
    )
    waws_d = nc.dram_tensor("waws", (L, WAW), bf16, kind="ExternalInput")
    xt_d = nc.dram_tensor("xt", (L, BLOC, N), bf16, kind="ExternalInput")
    # bnk = [(bs/2).f32-bits | 0.5*(K>0) f32-bits in [p, (g,k,u)]] (128, 65)
    bnk_d = nc.dram_tensor("bnk", (128, 1 + BLOC * NBLK), i32, kind="ExternalInput")
    y2_d = nc.dram_tensor("y2", (128, NCH * BLOC), bf16, kind="ExternalInput")
    out_d = nc.dram_tensor(
        "out", (128, NGRP, GSZ * NBLK), f32, kind="ExternalOutput"
    )

    with tile.TileContext(nc) as tc:
        with (
            tc.tile_pool(name="const", bufs=1) as constp,
            tc.tile_pool(name="xtp", bufs=1) as xtp,
            tc.tile_pool(name="ep", bufs=33) as ep,
            tc.tile_pool(name="tsp", bufs=2) as tsp,
            tc.tile_pool(name="finp", bufs=1) as finp,
            tc.tile_pool(name="lgp", bufs=3, space="PSUM") as lgp,
            tc.tile_pool(name="o2p", bufs=1, space="PSUM") as o2p,
        ):
            # ---- preload the ACT exp table during the DMA prologue ----
            dum = constp.tile([128, 1], f32, name="dum")
            nc.vector.memset(dum[:], 0.0)
            dum2 = constp.tile([128, 1], f32, name="dum2")
            nc.scalar.activation(dum2[:], dum[:], Exp)

            # ---- loads (wa half1 first: it gates the first matmul) ----
            waws_sb = constp.tile([L, WAW], bf16, name="waws")
            nc.scalar.dma_start(waws_sb[:, 0:HALF], waws_d[:, 0:HALF])
            nc.scalar.dma_start(waws_sb[:, HALF:], waws_d[:, HALF:])
            bnk_sb = constp.tile([128, 1 + BLOC * NBLK], i32, name="bnk")
            nc.scalar.dma_start(bnk_sb[:], bnk_d[:])
            # y2all: cols 0..31 = y per (b,c) bf16, cols 32..63 = 1.0
            y2all = constp.tile([128, 2 * NCH * BLOC], bf16, name="y2all")
            nc.scalar.dma_start(y2all[:, 0 : NCH * BLOC], y2_d[:])
            nc.vector.memset(y2all[:, NCH * BLOC :], 1.0)
            y2v = y2all[:].rearrange("p (two k) -> p k two", two=2)

            xt_sb = xtp.tile([L, BLOC, N], bf16, name="xt")
            for lo, hi in ((0, 1), (1, 2), (2, 4), (4, 6), (6, 8)):
                nc.sync.dma_start(xt_sb[:, lo:hi, :], xt_d[:, lo:hi, :])

            wa_sb = waws_sb[:, 0:M]
            sel_sb = waws_sb[:, M + 2 : M + 2 + SELC]
            bh_sb = bnk_sb[:, 0:1].bitcast(f32)  # bs/2 per partition
            kh_sb = bnk_sb[:, 1 : 1 + BLOC * NBLK].bitcast(f32)  # 0.5*(K>0)

            # ---- PE warmup during the DMA wait: ~2.6us of dummy matmuls
            # flips the HAM clock gate to 2.4GHz before real work. Warmups
            # cycle through the lg pool buffers (no readers -> PE-only WAW) ----
            wsrc = constp.tile([128, HALF], bf16, name="wsrc")
            nc.vector.memset(wsrc[:], 0.0)
            for _ in range(6):
                warm = lgp.tile([128, M], f32, tag="lg")
                nc.tensor.matmul(
                    warm[:, 0:HALF], wsrc[:, 0:128], wsrc[:], skip_group_check=True
                )

            es = {}

            def mm1_chunk(b, c):
                tile_id = NCH * b + c
                xt_c = xt_sb[:, b, c * 128 : (c + 1) * 128]
                lg = lgp.tile([128, M], f32, tag="lg")
                nc.tensor.matmul(lg[:, 0:HALF], xt_c, wa_sb[:, 0:HALF])
                nc.tensor.matmul(lg[:, HALF:M], xt_c, wa_sb[:, HALF:M])
                e_c = ep.tile([128, M], bf16, tag="e")
                if tile_id == 31:
                    # split the last tile across both engines (latency)
                    nc.scalar.activation(e_c[:, 0:HALF], lg[:, 0:HALF], Exp)
                    nc.vector.tensor_scalar(
                        e_c[:, HALF:M].bitcast(i16),
                        lg[:, HALF:M],
                        SCH_A,
                        SCH_B,
                        MULT,
                        ADD,
                    )
                elif tile_id % 2 == 1 and tile_id != 1:
                    nc.vector.tensor_scalar(
                        e_c[:].bitcast(i16), lg[:], SCH_A, SCH_B, MULT, ADD
                    )
                else:
                    nc.scalar.activation(e_c[:], lg[:], Exp)
                es[(b, c)] = e_c

            def mm1_batch(b):
                for c in range(NCH):
                    mm1_chunk(b, c)

            def mm2_group(g, interleave=None):
                # two single-bank PSUM tiles (one per m-half) so the two
                # PSUM->SBUF copies have disjoint sources and run in parallel
                o2a = o2p.tile([128, HALF], f32, tag="o2a")
                o2b = o2p.tile([128, HALF], f32, tag="o2b")
                halves = [o2a, o2b]
                inter = list(interleave) if interleave else []
                for h in range(2):
                    for c in range(NCH):
                        for j in range(GSZ):
                            b = g * GSZ + j
                            nc.tensor.matmul(
                                halves[h][32 * j : 32 * j + 2, :],
                                y2v[:, NCH * b + c, :],
                                es[(b, c)][:, h * HALF : (h + 1) * HALF],
                                start=(c == 0),
                                stop=(c == NCH - 1),
                                skip_group_check=True,
                                tile_position=(0, 32 * j),
                            )
                        # keep lg production flowing during this mm2 burst
                        if inter:
                            inter.pop(0)()
                ts_a = tsp.tile([128, HALF], bf16, tag="tsa")
                ts_b = tsp.tile([128, HALF], bf16, tag="tsb")
                nc.vector.tensor_copy(ts_a[:], o2a[:])
                nc.scalar.copy(ts_b[:], o2b[:])
                return ts_a, ts_b

            def transpose_group(g, ts_pair):
                # out[p, r] = ts[row(r), 128k+p] via lhsT=ts block, rhs=sel.
                # Output reuses the (now dead) o2a bank; blocks 0-3 need only
                # ts_a so they start while ts_b still copies.
                ts_a, ts_b = ts_pair
                tr = o2p.tile([128, HALF], f32, tag="o2a")
                for k in range(NBLK):
                    src = ts_a if k < 4 else ts_b
                    nc.tensor.matmul(
                        tr[:, SELC * k : SELC * (k + 1)],
                        src[:, 128 * (k % 4) : 128 * (k % 4 + 1)],
                        sel_sb,
                        skip_group_check=True,
                    )
                return tr

            def epilogue_group(g, tr):
                # transposed region: [128, (k, u, v)] v=0 -> t, v=1 -> s
                mg = tr[:, 0 : SELC * NBLK].rearrange(
                    "p (k u v) -> p k u v", u=GSZ, v=2
                )
                W = GSZ * NBLK  # 32
                srec = finp.tile([128, W], f32, tag=f"sr{g}", name=f"sr{g}")
                srv = srec[:].rearrange("p (k u) -> p k u", u=GSZ)
                nc.vector.reciprocal(srv, mg[:, :, :, 1])
                rr = finp.tile([128, W], f32, tag=f"rr{g}", name=f"rr{g}")
                rrv = rr[:].rearrange("p (k u) -> p k u", u=GSZ)
                nc.vector.tensor_mul(rrv, mg[:, :, :, 0], srv)
                # d = 0.5*(1 + tanh(r/2 + bs/2)); tanh is in the exp table set
                th = finp.tile([128, W], f32, tag=f"th{g}", name=f"th{g}")
                nc.scalar.activation(th[:], rr[:], Tanh, bias=bh_sb, scale=0.5)
                th1 = finp.tile([128, W], f32, tag=f"t1{g}", name=f"t1{g}")
                nc.vector.tensor_scalar_add(th1[:], th[:], 1.0)
                dm = finp.tile([128, W], f32, tag=f"dm{g}", name=f"dm{g}")
                nc.vector.tensor_mul(dm[:], th1[:], kh_sb[:, g * W : (g + 1) * W])
                nc.sync.dma_start(out_d[:, g, :], dm[:])

            # ---- schedule: keep PE fed; b6 chunks interleave into mm2_g0's
            # quad burst so lg tiles keep flowing to the exp engines ----
            for b in range(6):
                mm1_batch(b)
            ts0 = mm2_group(
                0, interleave=[lambda c=c: mm1_chunk(6, c) for c in range(NCH)]
            )
            mm1_batch(7)
            tr0 = transpose_group(0, ts0)
            epilogue_group(0, tr0)
            ts1 = mm2_group(1)
            tr1 = transpose_group(1, ts1)
            epilogue_group(1, tr1)

    import concourse.mybir as mybir_mod

    _dedupe_ldweights(nc, mybir_mod)
    nc.compile()
    return nc


def _get_nc():
    if "nc" not in _STATE:
        _STATE["nc"] = _build()
    return _STATE["nc"]


def _make_in_maps(X, K, Wa, Ws, bs):
    import ml_dtypes

    bf16 = ml_dtypes.bfloat16
    X = np.asarray(X, dtype=np.float32)
    K = np.ascontiguousarray(np.asarray(K, dtype=np.int32))
    Wa = np.asarray(Wa, dtype=np.float32)
    Ws = np.asarray(Ws, dtype=np.float32)
    bsv = float(np.asarray(bs, dtype=np.float32).reshape(-1)[0])

    waws = np.zeros((L, WAW), dtype=bf16)
    waws[:, 0:M] = Wa.astype(bf16)
    waws[:, M] = Ws.astype(bf16)
    for u in range(GSZ):
        for v in range(2):
            waws[32 * u + v, M + 2 + 2 * u + v] = 1.0

    bh = np.full((128, 1), 0.5 * bsv, dtype=np.float32)
    # y[b, n] = X[b] @ Ws in bf16 operands (like the device would)
    Xb = X.astype(bf16)
    yfull = Xb.reshape(-1, L).astype(np.float32) @ Ws.astype(bf16).astype(
        np.float32
    )
    yfull = yfull.reshape(B, N)

    in_maps = []
    for cid in range(NCORES):
        sl = slice(cid * BLOC, (cid + 1) * BLOC)
        xt = np.ascontiguousarray(np.transpose(Xb[sl], (2, 0, 1)))
        # K[b, m] with b = 4g+u, m = 128k+p  ->  kh[p, (g,k,u)] = 0.5*(K>0)
        kh = (
            (K[sl] > 0)
            .astype(np.float32)
            .reshape(NGRP, GSZ, NBLK, 128)
            .transpose(3, 0, 2, 1)
            .reshape(128, -1)
        ) * np.float32(0.5)
        bnk = np.concatenate(
            [bh.view(np.int32), np.ascontiguousarray(kh).view(np.int32)], axis=1
        )
        # y2[p, (b,c)] = y[b, c*128 + p] bf16
        y2 = np.ascontiguousarray(
            yfull[sl].reshape(BLOC, NCH, 128).transpose(2, 0, 1).reshape(128, -1)
        ).astype(bf16)
        in_maps.append(
            dict(xt=xt, waws=waws, bnk=np.ascontiguousarray(bnk), y2=y2)
        )
    return in_maps


def _run(X, K, Wa, Ws, bs, **spmd_kwargs):
    from concourse.bass_utils import run_bass_kernel_spmd

    nc = _get_nc()
    in_maps = _make_in_maps(X, K, Wa, Ws, bs)
    res = run_bass_kernel_spmd(
        nc, in_maps, core_ids=list(range(NCORES)), **spmd_kwargs
    )
    outs = []
    for r in res.results:
        o = r["out"]  # (128, g, (k,u)): out[p, g, k*4+u] = d[4g+u, 128k+p]
        outs.append(
            np.transpose(o.reshape(128, NGRP, NBLK, GSZ), (1, 3, 2, 0)).reshape(
                BLOC, M
            )
        )
    return np.ascontiguousarray(
        np.concatenate(outs, axis=0).astype(np.float32)
    ), res


def kernel(X, K, Wa, ba, Ws, bs):
    out, _ = _run(X, K, Wa, Ws, bs)
    return out


def kernel_traced(X, K, Wa, ba, Ws, bs):
    out, res = _run(X, K, Wa, Ws, bs, trace=False)
    return out, res
